# revision 1
# baseline (speedup 1.0000x reference)
"""Taylor feature map kernel for Trainium2 (Bass/Tile), 8-core SPMD.

Input  x:   (2, 16, 2048, 64) f32  -> 65536 rows of dim 64
Output out: (2, 16, 2048, 2145) f32 per row:
    [1, x/D^0.25, x_i^2/(sqrt(D)*sqrt(2)), x_i*x_j/sqrt(D) for i<j (row-major)]

Sharding: rows are purely elementwise -> split 65536 rows into 8 contiguous
chunks of 8192, one per NeuronCore. No communication.

Per-core layout: supertiles of G*128 rows (128 SBUF partitions x G row-groups
along the free dim). All feature blocks are computed straight into one
[128, G, 2145] SBUF tile which is stored with a single large DMA.
"""

import math
from contextlib import ExitStack

import numpy as np

try:
    import concourse.bass as bass
except ImportError:  # container path for the concourse framework
    import sys

    sys.path.insert(0, "/opt/trn_rl_repo")
    import concourse.bass as bass

import concourse.mybir as mybir
from concourse import tile
from concourse.bass_utils import run_bass_kernel_spmd
from concourse.vector_clock import ScopedClock

MAX_WAITS = 1


class SplitWaitTileContext(tile.TileContext):
    """The stock walrus in this environment rejects instructions carrying
    more than one sync wait ("Too many sync wait commands", observed for
    both TPB_CTRL Drain and DMA_DIRECT2D). Hoist excess waits onto NoOp
    carrier instructions committed just before, on the same engine queue."""

    def _split_waits(self, inst):
        si = getattr(inst, "sync_info", None)
        eng = getattr(inst, "engine", None)
        if (
            si is None
            or not si.on_wait
            or len(si.on_wait) <= MAX_WAITS
            or eng is None
            or eng == mybir.EngineType.Unassigned
        ):
            return None
        waits = list(si.on_wait)
        extra, keep = waits[:-MAX_WAITS], waits[-MAX_WAITS:]
        inst.sync_info = mybir.SyncInfo(on_wait=keep,
                                        on_update=list(si.on_update))
        nops = []
        for i in range(0, len(extra), MAX_WAITS):
            nops.append(mybir.InstNoOp(
                name=self.nc.get_next_instruction_name(),
                sync_info=mybir.SyncInfo(on_wait=extra[i:i + MAX_WAITS],
                                         on_update=[]),
                bass_nofuse=True,
                engine=eng,
            ))
        return nops

    def _commit_instruction(self, inst, lazy_reg_writes=True):
        if isinstance(inst, mybir.Instruction):
            nops = self._split_waits(inst)
            if nops:
                for nop in nops:
                    super()._commit_instruction(nop)
        return super()._commit_instruction(inst, lazy_reg_writes)

    def _drain_and_barrier(self, tick_clock, wait_clock):
        nc = self.nc
        drain_inst = nc.sync.drain()
        wait_clock.add_sem_waits(
            drain_inst.ins, ScopedClock({None: tick_clock.global_clock})
        )
        nops = self._split_waits(drain_inst.ins)
        if nops:
            # _commit path is closed here; append carriers directly, then
            # re-emit a drain that executes after them on the same queue.
            for nop in nops:
                self._add_instruction(nop)
            nc.sync.drain()

        nc.all_engine_barrier()
        assert self.sems is not None
        popped = nc._tile_sem_poison_stack.pop()
        assert popped is self._sem_poison
        nc.clear_and_free_semaphores(list(self.sems.allocated().values()))
        nc.all_engine_barrier()

D = 64
N_CROSS = (D * (D - 1)) // 2  # 2016
OUT_D = 1 + D + D + N_CROSS   # 2145
P = 128
N_CORES = 8
ROWS_TOTAL = 2 * 16 * 2048    # 65536
ROWS_PER_CORE = ROWS_TOTAL // N_CORES  # 8192

RD = math.sqrt(D)                      # 8.0
RRD_INV = 1.0 / math.sqrt(RD)          # 1/D^0.25; note (1/rrd)^2 == 1/rd
DIAG_C = 1.0 / math.sqrt(RD * math.sqrt(2.0))  # (c*x)^2 = x^2/(rd*sqrt2)

# engine bands for the 63 cross-product jobs (job i has run length 63-i):
# POOL takes [0, POOL_END) as paired ops, ACT takes [POOL_END, ACT_END) as
# per-(i,group) scale-copies, DVE takes [ACT_END, 63) as paired ops.
POOL_END = 16
G = 8  # row-groups per supertile

_OFF = [0] * 64
for _i in range(63):
    _OFF[_i + 1] = _OFF[_i] + (63 - _i)
BASE = 1 + 2 * D        # 129, start of cross block
SPLIT_COL = BASE + _OFF[POOL_END]  # 1017: POOL writes cols [129, SPLIT_COL),
                                   # DVE writes [SPLIT_COL, OUT_D) in its own
                                   # tile so the bands share no dep granules
B_COLS = OUT_D - SPLIT_COL         # 1128


def _pair_aps(a_sb, out_sb, groups, i, out_col0, out_w):
    """4D access patterns computing cross rows i and i+1 in one op.

    out[p,g,q,j] = y_{i+q} * y_{i+q+1+j},  q in {0,1}, j in [0, 63-i).
    Row i+1's run is padded by one garbage element which lands on
    off(i+2)[0] and is overwritten by the next op on the same engine.
    Reads y from tile a_sb (width SPLIT_COL); writes into out_sb
    (width out_w) at local column BASE+off(i)-out_col0.
    """
    n = 63 - i
    a_t = a_sb[:, :, 0:1]
    o_t = out_sb[:, :, 0:1]
    a_pstep = a_t.ap[0][0]
    o_pstep = o_t.ap[0][0]
    out = bass.AP(o_t.tensor, BASE + _OFF[i] - out_col0,
                  [[o_pstep, P], [out_w, groups], [n, 2], [1, n]])
    in0 = bass.AP(a_t.tensor, 1 + i,
                  [[a_pstep, P], [SPLIT_COL, groups], [1, 2], [0, n]])
    in1 = bass.AP(a_t.tensor, 2 + i,
                  [[a_pstep, P], [SPLIT_COL, groups], [1, 2], [1, n]])
    return out, in0, in1


def build_nc(rows_per_core: int = ROWS_PER_CORE, groups: int = G) -> bass.Bass:
    n_super = rows_per_core // (groups * P)
    assert n_super * groups * P == rows_per_core

    nc = bass.Bass()
    x = nc.declare_dram_parameter("x", [rows_per_core, D], mybir.dt.float32,
                                  isOutput=False)
    out = nc.declare_dram_parameter("out", [rows_per_core, OUT_D],
                                    mybir.dt.float32, isOutput=True)

    f32 = mybir.dt.float32
    rows_st = groups * P
    mult = mybir.AluOpType.mult

    with SplitWaitTileContext(nc) as tc, ExitStack() as ctx:
        xp = ctx.enter_context(tc.tile_pool(name="xp", bufs=n_super))
        op = ctx.enter_context(tc.tile_pool(name="op", bufs=2))

        # prefetch the whole input up front on the ACT HWDGE ring
        # (row r = p*groups + g -> 2KB contiguous per partition per tile)
        x_tiles = []
        for st in range(n_super):
            x_view = x[st * rows_st:(st + 1) * rows_st, :]
            x_sb = xp.tile([P, groups, D], f32)
            nc.scalar.dma_start(x_sb[:],
                                x_view.rearrange("(p g) d -> p g d", g=groups))
            x_tiles.append(x_sb)

        for st in range(n_super):
            x_sb = x_tiles[st]
            a_sb = op.tile([P, groups, SPLIT_COL], f32, tag="a")
            b_sb = op.tile([P, groups, B_COLS], f32, tag="b")
            # ones column (POOL)
            nc.gpsimd.memset(a_sb[:, :, 0:1], 1.0)
            # linear block: y = x / D^0.25  (cols 1..65, DVE)
            nc.vector.tensor_scalar_mul(a_sb[:, :, 1:1 + D], x_sb[:], RRD_INV)
            # diag block: (x*c2)*x = x^2/(rd*sqrt2)  (cols 65..129, ACT)
            nc.scalar.activation(a_sb[:, :, 1 + D:1 + 2 * D], x_sb[:],
                                 mybir.ActivationFunctionType.Square,
                                 scale=DIAG_C)

            # cross block, POOL band in tile A: pairs 0..13, singles 14, 15
            # (the last jobs stay single so no garbage spills into tile B)
            i = 0
            while i < POOL_END:
                if i + 3 < POOL_END:
                    o_ap, a_ap, b_ap = _pair_aps(a_sb, a_sb, groups, i,
                                                 0, SPLIT_COL)
                    nc.gpsimd.tensor_mul(o_ap, a_ap, b_ap)
                    i += 2
                else:
                    n = 63 - i
                    dst = a_sb[:, :, BASE + _OFF[i]: BASE + _OFF[i] + n]
                    a = a_sb[:, :, 1 + i: 2 + i].broadcast_to((P, groups, n))
                    nc.gpsimd.tensor_mul(dst, a, a_sb[:, :, 2 + i: 2 + i + n])
                    i += 1

            # DVE band in tile B: pairs (16,17)..(60,61), single 62
            i = POOL_END
            while i < 63:
                if i + 1 < 62:
                    o_ap, a_ap, b_ap = _pair_aps(a_sb, b_sb, groups, i,
                                                 SPLIT_COL, B_COLS)
                    nc.vector.tensor_mul(o_ap, a_ap, b_ap)
                    i += 2
                else:
                    n = 63 - i
                    c0 = BASE + _OFF[i] - SPLIT_COL
                    dst = b_sb[:, :, c0: c0 + n]
                    a = a_sb[:, :, 1 + i: 2 + i].broadcast_to((P, groups, n))
                    nc.vector.tensor_mul(dst, a, a_sb[:, :, 2 + i: 2 + i + n])
                    i += 1

            rview = out[st * rows_st:(st + 1) * rows_st, :]
            nc.sync.dma_start(
                rview[:, 0:SPLIT_COL].rearrange("(p g) d -> p g d", g=groups),
                a_sb[:])
            nc.sync.dma_start(
                rview[:, SPLIT_COL:OUT_D].rearrange("(p g) d -> p g d",
                                                    g=groups),
                b_sb[:])
    return nc


_NC_CACHE: dict = {}


def _install_ntff_hook_shim():
    """The image's antenv lacks axon_hooks; provide it so trace=True can
    drive NRT profiling via ctypes into libaxon_pjrt.so."""
    import sys as _sys
    import types
    import ctypes
    import contextlib

    if "antenv.axon_hooks" in _sys.modules:
        return
    so_path = "/opt/axon/libaxon_pjrt.so"
    lib = ctypes.CDLL(so_path)
    if not hasattr(lib, "axon_start_nrt_profile"):
        return
    lib.axon_start_nrt_profile.argtypes = [
        ctypes.POINTER(ctypes.c_int64), ctypes.c_size_t]
    lib.axon_start_nrt_profile.restype = ctypes.c_int64
    lib.axon_stop_nrt_profile.argtypes = [ctypes.c_char_p]
    lib.axon_stop_nrt_profile.restype = ctypes.c_int64

    @contextlib.contextmanager
    def _hook(output_dir, device_ids):
        import jax
        jax.devices()
        if device_ids:
            ids = (ctypes.c_int64 * len(device_ids))(*device_ids)
            rc = lib.axon_start_nrt_profile(ids, len(device_ids))
        else:
            rc = lib.axon_start_nrt_profile(None, 0)
        if rc != 0:
            raise RuntimeError(f"axon_start_nrt_profile rc={rc}")
        try:
            yield
        finally:
            n = lib.axon_stop_nrt_profile(str(output_dir).encode())
            print(f"ntff profile: {n} file(s) written to {output_dir}")

    mod = types.ModuleType("antenv.axon_hooks")
    mod.set_axon_ntff_profile_hook = lambda h: None
    mod.get_axon_ntff_profile_hook = lambda: _hook
    _sys.modules["antenv.axon_hooks"] = mod
    import antenv
    antenv.axon_hooks = mod


def _get_nc():
    if "nc" not in _NC_CACHE:
        _NC_CACHE["nc"] = build_nc()
    return _NC_CACHE["nc"]


def _install_loud_cc_hook():
    """Surface the real python traceback when the PJRT compile callback
    fails (the C++ caller swallows it)."""
    from concourse import bass2jax
    bass2jax.install_neuronx_cc_hook()
    try:
        import libneuronxla
    except ImportError:
        return
    if getattr(libneuronxla, "_loud_wrapped", False):
        return
    orig = libneuronxla.neuronx_cc

    def loud_hook(*a, **kw):
        try:
            return orig(*a, **kw)
        except BaseException:
            import traceback
            import sys as _s
            traceback.print_exc()
            _s.stderr.flush()
            raise

    libneuronxla.neuronx_cc = loud_hook
    libneuronxla._loud_wrapped = True
    bass2jax.install_neuronx_cc_hook = lambda: None


def _run(x_np: np.ndarray, trace: bool = False):
    _install_loud_cc_hook()
    if trace:
        _install_ntff_hook_shim()
    nc = _get_nc()
    in_maps = [{"x": x_np[c * ROWS_PER_CORE:(c + 1) * ROWS_PER_CORE]}
               for c in range(N_CORES)]
    res = run_bass_kernel_spmd(nc, in_maps, list(range(N_CORES)), trace=trace)
    out = np.concatenate([res.results[c]["out"] for c in range(N_CORES)],
                         axis=0)
    return out, res


def kernel(x) -> np.ndarray:
    x_np = np.ascontiguousarray(np.asarray(x), dtype=np.float32)
    shape = x_np.shape
    x_np = x_np.reshape(ROWS_TOTAL, D)
    out, _ = _run(x_np, trace=False)
    return out.reshape(*shape[:-1], OUT_D)

